# revision 27
# baseline (speedup 1.0000x reference)
"""AreaAttention Trainium2 kernel: 8-core data-parallel over batch.

Each core processes one [512, 64, 64] image through:
  qk = SiLU(BN(conv1x1(x)));  v = SiLU(BN(conv1x1(x)))
  pp = SiLU(BN(conv3x3(v)))
  area attention (4 windows of 1024 tokens, 8 heads of dim 64) over qk/v
  y = SiLU(BN(conv1x1(attn_out + pp)))

Host prep: BN scales folded into conv weights, weights pre-transposed to
[K, M] lhsT layouts, everything pre-cast to bf16 so each tensor is a
single contiguous DMA. All matmuls bf16 with fp32 PSUM.

Attention uses PE row tiling throughout: S^T = K^T Q has a 64-deep
contraction, so head pairs (partitions 0-63 / 64-127 of a [128, NW]
q/k slab) run as concurrent (0,0)/(64,0) PE tiles; attn@V splits its
128-key contraction into two concurrent 64-key tiles accumulating into
separate PSUM banks, merged by one DVE add (the ones-column in Vaug
rides along in both tiles, so row 64 of the merged result is the
softmax denominator). Reciprocal is computed in a [128, 8] reshape and
broadcast across partitions via a DRAM bounce.

conv3x3 runs on flat [C, 4096] maps with shifted contiguous slices per
tap; two copies of v with boundary columns zeroed handle the horizontal
pad, range-clipping the vertical pad. Its matmuls are emitted as small
fill units interleaved into the attention stream (engine queues are
in-order, so this is what keeps PE busy during the exp chain), with
PSUM evicted by DVE to bf16 staging; the bias+SiLU runs as one batched
ACT stretch per window so the scalar engine never ping-pongs between
the exp and silu table sets mid-window (a set switch costs ~2.7us).
"""

import numpy as np

import concourse.bacc as bacc
import concourse.bass as bass
from concourse import mybir
from concourse.tile import TileContext
from concourse.masks import make_identity
from concourse.tile import add_dep_helper

P = 128
C = 512
CI = C // P          # 4 input-channel chunks
OCQK = 2 * C // P    # 8 qk output chunks
OC = C // P          # 4 output chunks
HW = 4096            # 64*64 tokens
NCH = HW // 512      # 8 n-chunks of 512
WIN = 4              # area windows
NW = HW // WIN       # 1024 tokens per window
HEADS = 8
HD = 64
JC = NW // P         # 8 key chunks of 128 per window
EPS = 1e-5
FP32 = mybir.dt.float32
BF16 = mybir.dt.bfloat16
SILU = mybir.ActivationFunctionType.Silu
EXP = mybir.ActivationFunctionType.Exp

# taps ordered center-first so a full-range matmul opens each PSUM group
TAPS = [(1, 1)] + [(ky, kx) for ky in range(3) for kx in range(3) if (ky, kx) != (1, 1)]
FILL_CHUNK = 6       # conv matmuls emitted per interleave point


class FillQueue:
    """Deferred emit-closures used to pad the PE queue during attention."""

    def __init__(self):
        self.units = []

    def add(self, fn):
        self.units.append(fn)

    def emit(self, n):
        for _ in range(n):
            if not self.units:
                return
            self.units.pop(0)()

    def drain(self):
        self.emit(len(self.units))


def _phase_a(nc, env):
    """v conv -> v_mid (SBUF) and qk conv -> qk_dram, nch-ordered."""
    xpool = env['xpool']; stpool = env['stpool']; psum_mm = env['psA4']
    x_ext = env['x_ext']; v_bf = env['v_bf']; qk_bf = env['qk_bf']
    v_mid = env['v_mid']; bv_sb = env['bv_sb']; bqk_sb = env['bqk_sb']
    qk_dram = env['qk_dram']

    for nch in range(NCH):
        xch = xpool.tile([P, CI, 512], BF16, tag="xch")
        nc.gpsimd.dma_start(out=xch[:], in_=x_ext[:, :, nch * 512 : (nch + 1) * 512])
        for oc in range(OC):
            ps = psum_mm.tile([P, 512], FP32, tag="mm")
            for ci in range(CI):
                nc.tensor.matmul(
                    ps[:],
                    v_bf[:, ci, oc * P : (oc + 1) * P],
                    xch[:, ci, :],
                    start=(ci == 0),
                    stop=(ci == CI - 1),
                )
            nc.scalar.activation(
                v_mid[:, oc, nch * 512 : (nch + 1) * 512], ps[:], SILU,
                bias=bv_sb[:, oc : oc + 1],
            )
        for oc in range(OCQK):
            ps = psum_mm.tile([P, 512], FP32, tag="mm")
            for ci in range(CI):
                nc.tensor.matmul(
                    ps[:],
                    qk_bf[:, ci, oc * P : (oc + 1) * P],
                    xch[:, ci, :],
                    start=(ci == 0),
                    stop=(ci == CI - 1),
                )
            st = stpool.tile([P, 512], BF16, tag="st")
            si = nc.scalar.activation(st[:], ps[:], SILU, bias=bqk_sb[:, oc : oc + 1])
            env['act_anchor'] = si
            nc.sync.dma_start(
                out=qk_dram[oc * P : (oc + 1) * P, nch * 512 : (nch + 1) * 512],
                in_=st[:],
            )


def _emit_win_setup(nc, env, g):
    """Transposes into vaug + halo copies for window g. Returns (vaug, vtap)."""
    psum_mm = env['psum_mm']; vaugp = env['vaugp']; vlr = env['vlr']
    v_mid = env['v_mid']; ident = env['ident']

    vaug = vaugp.tile([P, JC, HEADS, HD + 1], BF16, tag="vaug")
    nc.vector.memset(vaug[:, :, :, HD : HD + 1], 1.0)
    for jc in range(JC):
        t0 = g * NW + jc * P
        pt4 = psum_mm.tile([P, 512], BF16, tag="mm")
        for ci in range(CI):
            nc.tensor.matmul(
                pt4[:, ci * P : (ci + 1) * P],
                v_mid[:, ci, t0 : t0 + P],
                ident[:],
                is_transpose=True,
                start=(ci == 0),
                stop=(ci == CI - 1),
                skip_group_check=True,
            )
        nc.vector.tensor_copy(
            vaug[:, jc, :, 0:HD],
            pt4[:].rearrange("p (h d) -> p h d", h=HEADS),
        )

    og = NW * g - 128
    ca, cb = max(0, og), min(HW, NW * (g + 1) + 128)
    v_l = vlr.tile([P, CI, NW + 256], BF16, tag="vl")
    v_r = vlr.tile([P, CI, NW + 256], BF16, tag="vr")
    nc.vector.tensor_copy(v_l[:, :, ca - og : cb - og], v_mid[:, :, ca:cb])
    nc.vector.tensor_copy(v_r[:, :, ca - og : cb - og], v_mid[:, :, ca:cb])
    rl = v_l[:].rearrange("p c (r w) -> p c r w", w=64)
    rr = v_r[:].rearrange("p c (r w) -> p c r w", w=64)
    nc.vector.memset(rl[:, :, (ca - og) // 64 : (cb - og) // 64, 63:64], 0)
    nc.vector.memset(rr[:, :, (ca - og) // 64 : (cb - og) // 64, 0:1], 0)
    return vaug, {0: v_l, 1: None, 2: v_r}


def _build_conv3_fill(nc, env, g, vtap, og, fill):
    """Queue conv3x3 matmul units for window g into the fill queue."""
    psum_mm = env['psum_mm']; pepool = env['pepool']; pp_raw = env['pp_raw']
    v_mid = env['v_mid']; pew_ext = env['pew_ext']

    pe_tiles = {}

    def load_pe(oc):
        t = pepool.tile([P, CI * 9 * P], BF16, tag="pe")
        nc.gpsimd.dma_start(out=t[:], in_=pew_ext[oc])
        pe_tiles[oc] = t

    load_pe(0)
    for oc in range(OC):
        for nloc in range(2):
            nch = 2 * g + nloc
            n0 = nch * 512
            mms = []
            for ky, kx in TAPS:
                s = (ky - 1) * 64 + (kx - 1)
                lo = max(0, -s - n0)
                hi = min(512, HW - s - n0)
                for ci in range(CI):
                    if kx == 1:
                        rsl = (ci, None, n0 + s + lo, n0 + s + hi)
                    else:
                        rsl = (ci, kx, n0 + s + lo - og, n0 + s + hi - og)
                    mms.append((lo, hi, ky * 3 + kx, rsl))

            n_units = (len(mms) + FILL_CHUNK - 1) // FILL_CHUNK
            for u in range(n_units):
                lo_i = u * FILL_CHUNK
                hi_i = min(len(mms), lo_i + FILL_CHUNK)

                def unit(oc=oc, nloc=nloc, nch=nch, lo_i=lo_i, hi_i=hi_i,
                         mms=mms, first=(u == 0), last=(u == n_units - 1)):
                    key = (oc, nloc)
                    if first:
                        ps = psum_mm.tile([P, 512], FP32, tag="mm")
                        _conv_state[key] = ps
                        if nloc == 0 and oc + 1 < OC:
                            load_pe(oc + 1)
                    ps = _conv_state[key]
                    pe_v = pe_tiles[oc][:].rearrange(
                        "p (c t o) -> p c t o", c=CI, t=9
                    )
                    for i in range(lo_i, hi_i):
                        lo, hi, tap, (ci, kx, a, b) = mms[i]
                        if kx is None:
                            rsl = v_mid[:, ci, a:b]
                        else:
                            rsl = vtap[kx][:, ci, a:b]
                        nc.tensor.matmul(
                            ps[:, lo:hi],
                            pe_v[:, ci, tap, :],
                            rsl,
                            start=(i == 0),
                            stop=(i == len(mms) - 1),
                            skip_group_check=True,
                        )
                    if last:
                        nc.vector.tensor_copy(
                            pp_raw[:, oc, nloc * 512 : (nloc + 1) * 512], ps[:]
                        )
                        del _conv_state[key]

                fill.add(unit)


_conv_state = {}


def _emit_window(nc, env, g, vaug, fill, next_setup):
    """Attention for window g with conv fill, then silu/pr epilogue."""
    psS = env['psS']; psO = env['psO']; psum_mm = env['psum_mm']
    aexpp = env['aexpp']; qkp = env['qkp']; srowp = env['srowp']
    recipp = env['recipp']; tmpp = env['tmpp']; attnw = env['attnw']
    dram2 = env['dram2']; qk_dram = env['qk_dram']
    pp_raw = env['pp_raw']; y_raw = env['y_raw']; zpool = env['zpool']
    pr_bf = env['pr_bf']; bpe_sb = env['bpe_sb']
    bpr_sb = env['bpr_sb']; out_ext = env['out_ext']

    attn_w = attnw.tile([P, OC, NW], BF16, tag="attnw")

    for pair in range(HEADS // 2):
        q2 = qkp.tile([P, NW], BF16, tag="q2")
        nc.gpsimd.dma_start(
            out=q2[:],
            in_=qk_dram[pair * P : (pair + 1) * P, g * NW : (g + 1) * NW],
        )
        k2 = qkp.tile([P, NW], BF16, tag="k2")
        nc.gpsimd.dma_start(
            out=k2[:],
            in_=qk_dram[C + pair * P : C + (pair + 1) * P, g * NW : (g + 1) * NW],
        )

        a_tiles = {}
        for jc in range(JC):
            # one 4-bank tile for the head pair so both row-tiled matmuls
            # carry identical slot waits and can issue in lockstep
            ps2 = psS.tile([P, 2 * NW], FP32, tag="s")
            for half in range(2):
                hsl = slice(half * 512, (half + 1) * 512)
                nc.tensor.matmul(
                    ps2[:, hsl],
                    k2[0:HD, jc * P : (jc + 1) * P],
                    q2[0:HD, hsl],
                    start=True, stop=True,
                )
                nc.tensor.matmul(
                    ps2[:, NW + half * 512 : NW + (half + 1) * 512],
                    k2[HD:P, jc * P : (jc + 1) * P],
                    q2[HD:P, hsl],
                    start=True, stop=True,
                )
            a0 = aexpp.tile([P, NW], BF16, tag="aexp")
            e0 = nc.scalar.activation(a0[:], ps2[:, 0:NW], EXP, scale=0.125)
            a1 = aexpp.tile([P, NW], BF16, tag="aexp")
            e1 = nc.scalar.activation(a1[:], ps2[:, NW : 2 * NW], EXP, scale=0.125)
            anchor = env.get('act_anchor')
            if anchor is not None:
                add_dep_helper(e0.ins, anchor.ins, sync=False, reason="act-set batch")
                add_dep_helper(e1.ins, anchor.ins, sync=False, reason="act-set batch")
            env['last_exp'] = e1
            a_tiles[(0, jc)] = a0
            a_tiles[(1, jc)] = a1
            fill.emit(1)

        spair = srowp.tile([P, NW], FP32, tag="spair")
        dens = []
        for sub in range(2):
            h = 2 * pair + sub
            po = psO.tile([HD + 1, NW], FP32, tag="o")
            for jc in range(JC):
                a_t = a_tiles[(sub, jc)]
                for half in range(2):
                    hsl = slice(half * 512, (half + 1) * 512)
                    nc.tensor.matmul(
                        po[:, hsl],
                        vaug[:, jc, h, :],
                        a_t[:, hsl],
                        start=(jc == 0), stop=(jc == JC - 1),
                        skip_group_check=True,
                    )
            # both heads' numerators into one [128, NW] tile; denominator
            # rows staged to partition-0 tiles for the GpSimd broadcast
            nc.vector.tensor_copy(spair[sub * HD : (sub + 1) * HD, :], po[0:HD, :])
            den = recipp.tile([1, NW], FP32, tag=f"den{sub}")
            nc.vector.tensor_copy(den[:], po[HD : HD + 1, :])
            dens.append(den)
            fill.emit(1)

        # pair-level normalize: one full-width reciprocal and one multiply
        # writing straight into attn_w (the pair is exactly one oc slab)
        rbc = recipp.tile([P, NW], FP32, tag="rbc")
        nc.gpsimd.partition_broadcast(rbc[:], dens[1][:], channels=P)
        nc.gpsimd.partition_broadcast(rbc[0:HD, :], dens[0][:], channels=HD)
        rbr = recipp.tile([P, NW], FP32, tag="rbr")
        nc.vector.reciprocal_approx_fast(rbr[:], rbc[:])
        nc.vector.tensor_mul(attn_w[:, pair, :], spair[:], rbr[:])
        fill.emit(2)

        # next window's transposes + halos ahead of the last pair's
        # normalize chain in the DVE queue: they fill the boundary
        if pair == 2 and next_setup is not None:
            next_setup()

    fill.drain()

    # one batched ACT-silu stretch per window: this window's conv3x3
    # output plus the previous window's pr output (both staged in bf16),
    # so ACT stays in one table set per stretch
    last_exp = env.get('last_exp')
    for oc in range(OC):
        si = nc.scalar.activation(
            pp_raw[:, oc, :], pp_raw[:, oc, :], SILU, bias=bpe_sb[:, oc : oc + 1]
        )
        if last_exp is not None:
            add_dep_helper(si.ins, last_exp.ins, sync=False, reason="act-set batch")
        env['act_anchor'] = si
    if g > 0:
        _emit_y_silu_dma(nc, env, g - 1, last_exp)

    # z = attn + pp, then queue this window's pr conv as fill for the
    # next window's attention (evicted by DVE to y_raw); the last window's
    # pr runs in the epilogue instead
    prs = []
    zs = []
    for nloc in range(2):
        z = zpool.tile([P, CI, 512], BF16, tag="z")
        zs.append(z)
        nc.vector.tensor_add(
            z[:],
            attn_w[:, :, nloc * 512 : (nloc + 1) * 512],
            pp_raw[:, :, nloc * 512 : (nloc + 1) * 512],
        )
        for oc in range(OC):
            def pr_unit(nloc=nloc, oc=oc, z=z):
                ps = psum_mm.tile([P, 512], FP32, tag="mm")
                for ci in range(CI):
                    nc.tensor.matmul(
                        ps[:],
                        pr_bf[:, ci, oc * P : (oc + 1) * P],
                        z[:, ci, :],
                        start=(ci == 0),
                        stop=(ci == CI - 1),
                    )
                nc.vector.tensor_copy(
                    y_raw[:, oc, nloc * 512 : (nloc + 1) * 512], ps[:]
                )
            prs.append(pr_unit)
    if g < WIN - 1:
        env['pending_pr'] = prs
    else:
        env['z_last'] = zs


def _emit_y_silu_dma(nc, env, gw, last_exp):
    """SiLU (in place) + output DMA for window gw's staged pr output."""
    y_raw = env['y_raw']; bpr_sb = env['bpr_sb']; out_ext = env['out_ext']
    for oc in range(OC):
        si = nc.scalar.activation(
            y_raw[:, oc, :], y_raw[:, oc, :], SILU, bias=bpr_sb[:, oc : oc + 1]
        )
        if last_exp is not None:
            add_dep_helper(si.ins, last_exp.ins, sync=False, reason="act-set batch")
        env['act_anchor'] = si
        nc.sync.dma_start(
            out=out_ext[oc * P : (oc + 1) * P, gw * NW : (gw + 1) * NW],
            in_=y_raw[:, oc, :],
        )


def _build():
    nc = bacc.Bacc(None, target_bir_lowering=False, debug=False)

    x_ext = nc.declare_dram_parameter("x", [P, CI, HW], BF16, isOutput=False)
    qkw_ext = nc.declare_dram_parameter("qk_wt", [P, CI, 2 * C], BF16, isOutput=False)
    vw_ext = nc.declare_dram_parameter("v_wt", [P, CI, C], BF16, isOutput=False)
    pew_ext = nc.declare_dram_parameter("pe_wt", [OC, P, CI * 9 * P], BF16, isOutput=False)
    prw_ext = nc.declare_dram_parameter("pr_wt", [P, CI, C], BF16, isOutput=False)
    bqk_ext = nc.declare_dram_parameter("b_qk", [P, OCQK], FP32, isOutput=False)
    bv_ext = nc.declare_dram_parameter("b_v", [P, OC], FP32, isOutput=False)
    bpe_ext = nc.declare_dram_parameter("b_pe", [P, OC], FP32, isOutput=False)
    bpr_ext = nc.declare_dram_parameter("b_pr", [P, OC], FP32, isOutput=False)
    out_ext = nc.declare_dram_parameter("out", [C, HW], BF16, isOutput=True)

    with TileContext(nc) as tc:
        with (
            tc.tile_pool(name="const", bufs=1) as const_pool,
            tc.tile_pool(name="persist", bufs=1) as persist,
            tc.tile_pool(name="dram", bufs=1, space="DRAM") as dram,
            tc.tile_pool(name="dram2", bufs=3, space="DRAM") as dram2,
        ):
            ident = const_pool.tile([P, P], BF16)
            make_identity(nc, ident)

            v_bf = persist.tile([P, CI, C], BF16)
            nc.sync.dma_start(out=v_bf[:], in_=vw_ext[:])
            bv_sb = const_pool.tile([P, OC], FP32)
            nc.sync.dma_start(out=bv_sb[:], in_=bv_ext[:])
            bqk_sb = const_pool.tile([P, OCQK], FP32)
            nc.sync.dma_start(out=bqk_sb[:], in_=bqk_ext[:])
            qk_bf = persist.tile([P, CI, 2 * C], BF16)
            nc.sync.dma_start(out=qk_bf[:], in_=qkw_ext[:])
            bpe_sb = const_pool.tile([P, OC], FP32)
            nc.sync.dma_start(out=bpe_sb[:], in_=bpe_ext[:])
            bpr_sb = const_pool.tile([P, OC], FP32)
            nc.sync.dma_start(out=bpr_sb[:], in_=bpr_ext[:])
            pr_bf = persist.tile([P, CI, C], BF16)
            nc.sync.dma_start(out=pr_bf[:], in_=prw_ext[:])

            # v feature map (flat) -- attention V source and conv3x3 center
            v_mid = persist.tile([P, CI, HW], BF16)

            qk_dram = dram.tile([2 * C, HW], BF16)

            from contextlib import ExitStack
            env = dict(locals())

            # phase A runs on its own scoped pools; closing them returns
            # the space before the window pools open
            with ExitStack() as stack_a:
                for name, kw in [
                    ("psA4", dict(bufs=6, space="PSUM")),
                    ("xpool", dict(bufs=3)),
                    ("stpool", dict(bufs=6)),
                ]:
                    env[name] = stack_a.enter_context(
                        tc.tile_pool(name=name, **kw)
                    )
                _phase_a(nc, env)

            with ExitStack() as stack:
                pools = {}
                for name, kw in [
                    ("psum_mm", dict(bufs=2, space="PSUM")),
                    ("psS", dict(bufs=1, space="PSUM")),
                    ("psO", dict(bufs=1, space="PSUM")),
                    ("vlr", dict(bufs=1)),
                    ("attnw", dict(bufs=1)),
                    ("pepool", dict(bufs=2)),
                    ("vaugp", dict(bufs=2)),
                    ("aexpp", dict(bufs=16)),
                    ("qkp", dict(bufs=4)),
                    ("srowp", dict(bufs=2)),
                    ("recipp", dict(bufs=1)),
                    ("tmpp", dict(bufs=1)),
                    ("ppraw", dict(bufs=1)),
                    ("yraw", dict(bufs=1)),
                    ("zpool", dict(bufs=2)),
                ]:
                    pools[name] = stack.enter_context(
                        tc.tile_pool(name=name, **kw)
                    )
                env.update(pools)
                env['pp_raw'] = pools['ppraw'].tile([P, OC, NW], BF16, name='pp_raw', tag='pp_raw')
                env['y_raw'] = pools['yraw'].tile([P, OC, NW], BF16, name='y_raw', tag='y_raw')

                setups = {}

                def make_setup(g):
                    def setup():
                        vaug, vtap = _emit_win_setup(nc, env, g)
                        setups[g] = (vaug, vtap)
                    return setup

                make_setup(0)()
                for g in range(WIN):
                    vaug, vtap = setups[g]
                    fill = FillQueue()
                    for u in env.pop('pending_pr', []):
                        fill.add(u)
                    _build_conv3_fill(nc, env, g, vtap, NW * g - 128, fill)
                    nxt = make_setup(g + 1) if g + 1 < WIN else None
                    _emit_window(nc, env, g, vaug, fill, nxt)

                # epilogue: last window's pr conv, silu straight from
                # PSUM, bf16 out DMA (no staging hop on the critical tail)
                for nloc in range(2):
                    nch = 2 * (WIN - 1) + nloc
                    for oc in range(OC):
                        ps = pools['psum_mm'].tile([P, 512], FP32, tag="mm")
                        for ci in range(CI):
                            nc.tensor.matmul(
                                ps[:],
                                pr_bf[:, ci, oc * P : (oc + 1) * P],
                                env['z_last'][nloc][:, ci, :],
                                start=(ci == 0),
                                stop=(ci == CI - 1),
                            )
                        yst = pools['zpool'].tile([P, 512], BF16, tag="yst3", bufs=1)
                        nc.scalar.activation(
                            yst[:], ps[:], SILU, bias=bpr_sb[:, oc : oc + 1]
                        )
                        nc.sync.dma_start(
                            out=out_ext[oc * P : (oc + 1) * P,
                                        nch * 512 : (nch + 1) * 512],
                            in_=yst[:],
                        )

    nc.compile()
    return nc


_NC_CACHE = {}


def _get_nc():
    if "nc" not in _NC_CACHE:
        _NC_CACHE["nc"] = _build()
    return _NC_CACHE["nc"]


def _make_in_maps(inputs):
    import ml_dtypes

    bf16 = ml_dtypes.bfloat16
    x = np.asarray(inputs["x"], dtype=np.float32)          # [8, 512, 64, 64]
    B = x.shape[0]

    def fold(wname, gname, bname, mname, vname):
        g = np.asarray(inputs[gname], np.float32)
        b = np.asarray(inputs[bname], np.float32)
        m = np.asarray(inputs[mname], np.float32)
        v = np.asarray(inputs[vname], np.float32)
        s = g / np.sqrt(v + EPS)
        w = np.asarray(inputs[wname], np.float32)
        return s, (b - m * s).astype(np.float32), w

    s_qk, b_qk, qk_w = fold("qk_w", "qk_g", "qk_b", "qk_rm", "qk_rv")
    s_v, b_v, v_w = fold("v_w", "v_g", "v_b", "v_rm", "v_rv")
    s_pe, b_pe, pe_w = fold("pe_w", "pe_g", "pe_b", "pe_rm", "pe_rv")
    s_pr, b_pr, pr_w = fold("pr_w", "pr_g", "pr_b", "pr_rm", "pr_rv")

    def lhst(w_scaled, o_dim):
        # [O, C] scaled -> [128, CI, O] bf16 (partition = c % 128)
        wt = w_scaled.T.reshape(CI, P, o_dim).transpose(1, 0, 2)
        return np.ascontiguousarray(wt.astype(bf16))

    qk_wt = lhst(qk_w * s_qk[:, None], 2 * C)
    v_wt = lhst(v_w * s_v[:, None], C)
    pr_wt = lhst(pr_w * s_pr[:, None], C)

    # pe: [O, C, 3, 3] -> per oc chunk: [128(c%128), CI, 9, 128(o)] bf16
    pe = (pe_w * s_pe[:, None, None, None]).transpose(2, 3, 1, 0)  # ky,kx,c,o
    pe = pe.reshape(9, CI, P, OC, P)            # tap, ci, p, oc, op
    pe = pe.transpose(3, 2, 1, 0, 4)            # oc, p, ci, tap, op
    pe_wt = np.ascontiguousarray(pe.reshape(OC, P, CI * 9 * P).astype(bf16))

    def bias_r(b, n):
        return np.ascontiguousarray(b.reshape(n, P).T)

    shared = {
        "qk_wt": qk_wt, "v_wt": v_wt, "pe_wt": pe_wt, "pr_wt": pr_wt,
        "b_qk": bias_r(b_qk, OCQK), "b_v": bias_r(b_v, OC),
        "b_pe": bias_r(b_pe, OC), "b_pr": bias_r(b_pr, OC),
    }
    xs = x.reshape(B, CI, P, HW).transpose(0, 2, 1, 3).astype(bf16)
    return [
        {"x": np.ascontiguousarray(xs[i]), **shared}
        for i in range(B)
    ]


def kernel(**inputs):
    from concourse.bass_utils import run_bass_kernel_spmd

    in_maps = _make_in_maps(inputs)
    B = len(in_maps)
    nc = _get_nc()
    res = run_bass_kernel_spmd(nc, in_maps, core_ids=list(range(B)))
    out = np.stack([res.results[i]["out"] for i in range(B)], axis=0)
    return out.reshape(B, C, 64, 64).astype(np.float32)
